# revision 1
# baseline (speedup 1.0000x reference)
"""MoE (top-2 of 8 experts + shared expert) Trainium2 Bass kernel.

Strategy (expert-parallel, host-prepped routing, bf16 compute):
  - Router (sigmoid gate + top-2) runs on the host in fp32; it produces the
    token->expert gather lists.
  - Core c computes expert c's SwiGLU FFN densely over the tokens routed to
    it (padded to the max per-expert count), plus the shared-expert FFN over
    the token shard [c*1024, (c+1)*1024).
  - All matmuls are bf16 with 1024-wide moving operands (2-bank PSUM tiles)
    to amortize per-MM issue/LDWEIGHTS overhead; rel err ~4e-3 end to end.
  - The shared pass runs FIRST, streaming its weight slabs on the SP DMA
    ring, while the expert w1/w3 (11.5 MB bf16) preload into SBUF-resident
    tiles on the ACT DMA ring.  w2 slabs are streamed per chunk in both
    passes (SBUF does not fit all three resident at 1024-token chunks).
  - Host scatter-adds the per-expert outputs (scaled by combine weights)
    and the shared outputs into the final [8192, 2048] f32 result.

Everything on-device is feature-major ("K on partitions") so the x @ W.T
chains need no on-chip transposes:
  stage1:  h1T[m,:] = sum_k w1T[k, m].T @ xT[k, :]   (PSUM accum over k)
  g = silu(h1T) * h3T                                 (ACT + DVE, bf16 out)
  stage2:  yT[md,:] = sum_kh w2T[kh, md].T @ gT[kh,:]
"""

import os
import sys

for _p in ("/opt/trn_rl_repo", "/root/.axon_site/_ro/trn_rl_repo"):
    if os.path.isdir(_p) and _p not in sys.path:
        sys.path.insert(0, _p)

import numpy as np
import ml_dtypes

import concourse.bass as bass  # noqa: F401
import concourse.mybir as mybir
import concourse.tile as tile
from concourse import bacc
from concourse.bass_utils import run_bass_kernel_spmd

# Problem constants (hardcoded per spec)
N_TOK = 8192
D = 2048
H = 1408
E = 8
TOP_K = 2
ROUTE_SCALE = 1.0
P = 128
KD = D // P    # 16 k-tiles over D
MH = H // P    # 11 m-tiles over H
MD = D // P    # 16 m-tiles over D (stage 2 out)
SHARD = N_TOK // E  # 1024 shared-expert tokens per core

MAX_CHUNK = int(os.environ.get("MOE_MAX_CHUNK", "1536"))

F32 = mybir.dt.float32
BF16 = mybir.dt.bfloat16
NP_BF16 = ml_dtypes.bfloat16
SILU = mybir.ActivationFunctionType.Silu

LAST_RESULTS = None  # BassKernelResults of the most recent run (for test.py)

SKIP_MM = bool(os.environ.get("MOE_SKIP_MM"))
SKIP_DMA = bool(os.environ.get("MOE_SKIP_DMA"))
FIXED_W = bool(os.environ.get("MOE_FIXED_W"))  # timing probe: one lhsT for all MMs


def _enable_ldw_opt():
    """Turn the neuronxcc LDWEIGHTS optimization back on for our compiles.

    The environment's default flags carry --enable-ldw-opt=false inside
    --internal-backend-options; every bf16 matmul then pays a serialized
    ~53 ns weight load.  concourse.compiler_utils exposes the supported
    flag-override API; we rewrite just that one option.
    """
    if not os.environ.get("MOE_LDW_OPT"):
        return
    try:
        from concourse import compiler_utils
        flags = compiler_utils.get_compiler_flags()
        new = [f.replace("--enable-ldw-opt=false", "--enable-ldw-opt=true")
               for f in flags]
        if new != flags:
            compiler_utils.set_compiler_flags(new)
    except Exception:
        pass


_enable_ldw_opt()


def _chunks(T):
    """Split T (multiple of 128) into greedy chunks of <=MAX_CHUNK.

    Greedy (not balanced) minimizes the total matmul-instruction count:
    full-size chunks get full 512-col subs, and per-MM issue+LDWEIGHTS
    overhead dominates small-N matmuls.
    """
    sizes = [MAX_CHUNK] * (T // MAX_CHUNK)
    if T % MAX_CHUNK:
        sizes.append(T % MAX_CHUNK)
    assert sum(sizes) == T and all(s % 128 == 0 for s in sizes), sizes
    return sizes


def _subs(Tc):
    """Split Tc into matmul free-dim slices of <=512."""
    out = []
    rem = Tc
    while rem > 512:
        take = 384 if rem == 640 else 512
        out.append(take)
        rem -= take
    if rem:
        out.append(rem)
    s0 = 0
    res = []
    for s in out:
        res.append((s0, s))
        s0 += s
    return res


def _emit_ffn(nc, pools, x_dram, y_dram, T, get_w13, get_w2):
    """Emit one feature-major SwiGLU FFN over T tokens.

    get_w13(m) -> (w1_ap, w3_ap) each [P, KD*P]; get_w2(md) -> [P, MH*P].
    """
    xpool, gpool, spool, ypool, psum = pools

    fixed_w = [None]

    def lhs(ap):
        if not FIXED_W:
            return ap
        if fixed_w[0] is None:
            fixed_w[0] = ap
        return fixed_w[0]

    cs = 0
    for Tc in _chunks(T):
        # chunk of x on the ACT DMA ring (does not queue behind SP-ring
        # y writebacks), split into 4 k-group pieces so the first matmul
        # can start after ~1/4 of the transfer
        xt = xpool.tile([P, KD * Tc], BF16, name="xt")
        for k0 in range(0, KD, 4):
            SKIP_DMA or nc.scalar.dma_start(
                xt[:, k0 * Tc:(k0 + 4) * Tc].rearrange("p (k t) -> p k t", k=4),
                x_dram[k0:k0 + 4, :, cs:cs + Tc].rearrange("k p t -> p k t"),
            )
        x_tiles = [xt[:, k * Tc:(k + 1) * Tc] for k in range(KD)]
        subs = _subs(Tc)
        g_tiles = []
        for m in range(MH):
            w1m, w3m = get_w13(m)
            gm = gpool.tile([P, Tc], BF16, name=f"g{m}")
            ps1 = [psum.tile([P, 512], F32, name="acc")[:, :sl] for _, sl in subs]
            ps3 = [psum.tile([P, 512], F32, name="acc")[:, :sl] for _, sl in subs]
            for k in range(KD):
                # consecutive MMs share one lhsT so the PE skips the
                # per-MM weight reload
                w1k = lhs(w1m[:, k * P:(k + 1) * P])
                for j, (s0, sl) in enumerate(subs):
                    SKIP_MM or nc.tensor.matmul(
                        ps1[j], w1k, x_tiles[k][:, s0:s0 + sl],
                        start=(k == 0), stop=(k == KD - 1),
                    )
                w3k = lhs(w3m[:, k * P:(k + 1) * P])
                for j, (s0, sl) in enumerate(subs):
                    SKIP_MM or nc.tensor.matmul(
                        ps3[j], w3k, x_tiles[k][:, s0:s0 + sl],
                        start=(k == 0), stop=(k == KD - 1),
                    )
            for j, (s0, sl) in enumerate(subs):
                st = spool.tile([P, 512], BF16, name="silu")[:, :sl]
                SKIP_MM or nc.scalar.activation(st, ps1[j], SILU)
                SKIP_MM or nc.vector.tensor_mul(gm[:, s0:s0 + sl], st, ps3[j])
            g_tiles.append(gm)
        for md in range(MD):
            w2m = get_w2(md)
            ym = ypool.tile([P, Tc], BF16, name="ym")
            psy = [psum.tile([P, 512], F32, name="acc")[:, :sl] for _, sl in subs]
            for kh in range(MH):
                w2k = lhs(w2m[:, kh * P:(kh + 1) * P])
                for j, (s0, sl) in enumerate(subs):
                    SKIP_MM or nc.tensor.matmul(
                        psy[j], w2k, g_tiles[kh][:, s0:s0 + sl],
                        start=(kh == 0), stop=(kh == MH - 1),
                    )
            for j, (s0, sl) in enumerate(subs):
                SKIP_MM or nc.vector.tensor_copy(ym[:, s0:s0 + sl], psy[j])
            SKIP_MM or nc.sync.dma_start(y_dram[md, :, cs:cs + Tc], ym[:])
        cs += Tc


def _dedup_ldweights(nc):
    """Remove redundant PE Ldweights instructions.

    The legalizer inserts an InstLdweights before EVERY bf16 matmul, even
    when consecutive matmuls share one stationary operand.  The PE array
    keeps its weights across matmuls, so a reload of the exact same
    weights AP with only (non-transpose) matmuls in between is a no-op
    that still costs ~50 ns of serialized load time.  Drop those, keeping
    any Ldweights that carries semaphore waits/updates.
    """
    pe = mybir.EngineType.PE
    removed = 0
    for fn in nc.m.functions:
        for blk in fn.blocks:
            insts = blk.instructions
            keep = []
            last_key = None
            for inst in insts:
                if getattr(inst, "engine", None) == pe:
                    if isinstance(inst, mybir.InstLdweights):
                        ap = inst.ins[0]
                        key = (str(ap.memsetref), ap.offset, str(ap.ap),
                               str(ap.dtype), inst.is_transpose,
                               inst.tile_position, inst.perf_mode)
                        si = inst.sync_info
                        bare = si is None or (not si.on_wait and not si.on_update)
                        if bare and key == last_key:
                            removed += 1
                            continue
                        last_key = key
                    elif isinstance(inst, mybir.InstMatmult):
                        if inst.is_transpose:
                            last_key = None
                    else:
                        last_key = None
                keep.append(inst)
            if len(keep) != len(insts):
                blk.instructions = keep
    return removed


def _build_program(c_cap, loop_reps=1):
    nc = bacc.Bacc("TRN2", target_bir_lowering=False, debug=False, num_devices=E)
    xe = nc.dram_tensor("xe", [KD, P, c_cap], BF16, kind="ExternalInput").ap()
    xs = nc.dram_tensor("xs", [KD, P, SHARD], BF16, kind="ExternalInput").ap()
    # streamed bf16 slabs: expert + shared weights
    w1s = nc.dram_tensor("w1s", [MH, P, KD * P], BF16, kind="ExternalInput").ap()
    w3s = nc.dram_tensor("w3s", [MH, P, KD * P], BF16, kind="ExternalInput").ap()
    w2s = nc.dram_tensor("w2s", [MD, P, MH * P], BF16, kind="ExternalInput").ap()
    sw1s = nc.dram_tensor("sw1s", [MH, P, KD * P], BF16, kind="ExternalInput").ap()
    sw3s = nc.dram_tensor("sw3s", [MH, P, KD * P], BF16, kind="ExternalInput").ap()
    sw2s = nc.dram_tensor("sw2s", [MD, P, MH * P], BF16, kind="ExternalInput").ap()
    ye = nc.dram_tensor("ye", [MD, P, c_cap], BF16, kind="ExternalOutput").ap()
    ys = nc.dram_tensor("ys", [MD, P, SHARD], BF16, kind="ExternalOutput").ap()

    with tile.TileContext(nc) as tc:
        with tc.tile_pool(name="xpool", bufs=1) as xpool, \
             tc.tile_pool(name="wpool", bufs=3) as wpool, \
             tc.tile_pool(name="w2pool", bufs=4) as w2pool, \
             tc.tile_pool(name="gpool", bufs=1) as gpool, \
             tc.tile_pool(name="spool", bufs=3) as spool, \
             tc.tile_pool(name="ypool", bufs=3) as ypool, \
             tc.tile_pool(name="psum", bufs=8, space="PSUM") as psum:
            pools = (xpool, gpool, spool, ypool, psum)

            def body():
                def mk_w13(w1d, w3d):
                    def get(m):
                        w1m = wpool.tile([P, KD * P], BF16, name="w1m")
                        SKIP_DMA or nc.sync.dma_start(w1m[:], w1d[m])
                        w3m = wpool.tile([P, KD * P], BF16, name="w3m")
                        SKIP_DMA or nc.sync.dma_start(w3m[:], w3d[m])
                        return w1m[:], w3m[:]
                    return get

                def mk_w2(dram):
                    def get(md):
                        w2m = w2pool.tile([P, MH * P], BF16, name="w2m")
                        SKIP_DMA or nc.sync.dma_start(w2m[:], dram[md])
                        return w2m[:]
                    return get

                _emit_ffn(nc, pools, xs, ys, SHARD,
                          mk_w13(sw1s, sw3s), mk_w2(sw2s))
                _emit_ffn(nc, pools, xe, ye, c_cap,
                          mk_w13(w1s, w3s), mk_w2(w2s))

            if loop_reps > 1:
                with tc.For_i(0, loop_reps, 1):
                    body()
            else:
                body()
    nc.compile()
    if not os.environ.get("MOE_NO_LDW_DEDUP"):
        _dedup_ldweights(nc)
    return nc


def _tile_w13_stream(w):
    # [H, D] -> [MH, P, KD*P] with slab[m, p, k*P+j] = w[m*P+j, k*P+p]
    return np.ascontiguousarray(
        w.reshape(MH, P, KD, P).transpose(0, 3, 2, 1).reshape(MH, P, KD * P)
    )


def _tile_w2_stream(w):
    # [D, H] -> [MD, P, MH*P] with slab[md, p, kh*P+j] = w[md*P+j, kh*P+p]
    return np.ascontiguousarray(
        w.reshape(MD, P, MH, P).transpose(0, 3, 2, 1).reshape(MD, P, MH * P)
    )


def _tile_w13_res(w):
    # [H, D] -> [P, MH*KD*P] with t[p, (m*KD+k)*P+j] = w[m*P+j, k*P+p]
    return np.ascontiguousarray(
        w.reshape(MH, P, KD, P).transpose(3, 0, 2, 1).reshape(P, MH * KD * P)
    )


def _tile_x(xt):
    # [T, D] -> [KD, P, T]
    T = xt.shape[0]
    return np.ascontiguousarray(xt.reshape(T, KD, P).transpose(1, 2, 0))


def _untile_y(y):
    # [MD, P, T] -> [T, D]
    return y.transpose(2, 0, 1).reshape(y.shape[2], D).astype(np.float32)


def prepare(x, gate_w, expert_bias, w1, w2, w3, sw1, sw2, sw3):
    """Host routing + input prep. Returns (nc, in_maps, meta)."""
    x = np.ascontiguousarray(np.asarray(x, dtype=np.float32))
    gate_w = np.asarray(gate_w, dtype=np.float32)
    expert_bias = np.asarray(expert_bias, dtype=np.float32)
    w1 = np.asarray(w1, dtype=np.float32)
    w2 = np.asarray(w2, dtype=np.float32)
    w3 = np.asarray(w3, dtype=np.float32)
    sw1 = np.asarray(sw1, dtype=np.float32)
    sw2 = np.asarray(sw2, dtype=np.float32)
    sw3 = np.asarray(sw3, dtype=np.float32)

    # ---- host router (fp32, matches reference numerics) ----
    logits = x @ gate_w.T  # [N, E] f32
    scores = np.where(
        logits >= 0,
        1.0 / (1.0 + np.exp(-logits, dtype=np.float32)),
        np.exp(logits, dtype=np.float32) / (1.0 + np.exp(logits, dtype=np.float32)),
    ).astype(np.float32)
    biased = scores + expert_bias[None, :]
    i1 = np.argmax(biased, axis=1)
    tmp = biased.copy()
    tmp[np.arange(N_TOK), i1] = -np.inf
    i2 = np.argmax(tmp, axis=1)
    s1 = scores[np.arange(N_TOK), i1]
    s2 = scores[np.arange(N_TOK), i2]
    denom = s1 + s2 + np.float32(1e-20)
    c1 = (s1 / denom * np.float32(ROUTE_SCALE)).astype(np.float32)
    c2 = (s2 / denom * np.float32(ROUTE_SCALE)).astype(np.float32)

    idx_list, cw_list = [], []
    for e in range(E):
        m1 = i1 == e
        m2 = i2 == e
        idx = np.concatenate([np.nonzero(m1)[0], np.nonzero(m2)[0]])
        cw = np.concatenate([c1[m1], c2[m2]]).astype(np.float32)
        idx_list.append(idx)
        cw_list.append(cw)
    counts = [len(i) for i in idx_list]
    c_cap = max(512, -(-max(counts) // 128) * 128)

    # ---- build + compile the SPMD program for this capacity ----
    nc = _build_program(c_cap, loop_reps=int(os.environ.get("MOE_LOOP_REPS", "1")))

    # ---- per-core inputs ----
    x_bf = x.astype(NP_BF16)
    in_maps = []
    sw1s = _tile_w13_stream(sw1.astype(NP_BF16))
    sw3s = _tile_w13_stream(sw3.astype(NP_BF16))
    sw2s = _tile_w2_stream(sw2.astype(NP_BF16))
    for c in range(E):
        idx = idx_list[c]
        pad = c_cap - len(idx)
        idx_pad = np.concatenate([idx, np.zeros(pad, dtype=idx.dtype)]) if pad else idx
        in_maps.append({
            "xe": _tile_x(x_bf[idx_pad]),
            "xs": _tile_x(x_bf[c * SHARD:(c + 1) * SHARD]),
            "w1s": _tile_w13_stream(w1[c].astype(NP_BF16)),
            "w3s": _tile_w13_stream(w3[c].astype(NP_BF16)),
            "w2s": _tile_w2_stream(w2[c].astype(NP_BF16)),
            "sw1s": sw1s,
            "sw3s": sw3s,
            "sw2s": sw2s,
        })

    meta = (idx_list, cw_list, counts)
    return nc, in_maps, meta


def combine(meta, results):
    """Scatter-add per-core outputs into the final [N, D] array."""
    idx_list, cw_list, counts = meta
    out = np.zeros((N_TOK, D), dtype=np.float32)
    for c in range(E):
        r = results[c]
        cnt = counts[c]
        if cnt:
            y_tok = _untile_y(r["ye"])[:cnt]
            out[idx_list[c]] += cw_list[c][:, None] * y_tok
        out[c * SHARD:(c + 1) * SHARD] += _untile_y(r["ys"])
    return out


def kernel(x, gate_w, expert_bias, w1, w2, w3, sw1, sw2, sw3):
    nc, in_maps, meta = prepare(x, gate_w, expert_bias, w1, w2, w3, sw1, sw2, sw3)
    global LAST_RESULTS
    res = run_bass_kernel_spmd(nc, in_maps, core_ids=list(range(E)))
    LAST_RESULTS = res
    return combine(meta, res.results)



# revision 8
# speedup vs baseline: 1.8962x; 1.8962x over previous
"""MoE (top-2 of 8 experts + shared expert) Trainium2 Bass kernel.

Strategy (expert-parallel DENSE with on-device collectives, bf16):
  - Router (sigmoid gate + top-2) runs on the host in fp32; it produces a
    per-expert combine column over all 8192 tokens (zero for non-selected
    tokens).
  - Each core holds ONE expert's w1/w2/w3 plus a 2-tile (256-row, zero
    padded) H-slice of the shared-expert weights, and its own 1024-token
    shard of x.  On device: chunked AllGather of x -> every core runs its
    expert's SwiGLU FFN densely over all 8192 tokens, scales the gated
    hidden g by the combine column (zero for unrouted tokens), adds the
    shared-expert H-slice partial into the same stage-2 accumulation, and
    a chunked ReduceScatter(add) sums the 8 per-core contributions and
    hands each core its final 1024-token output shard.
  - Host->device IO is ~224 MB/call (vs ~487 MB for the gather/scatter
    baseline): per core x-shard 4.2 MB + expert weights 17.3 MB + shared
    slice 3.2 MB + combine 16 KB in, 4.2 MB out.
  - Everything on-chip is feature-major ("K on partitions") so x @ W.T
    chains need no transposes.  AG/RS are chunked (4 x 2048 tokens) so
    collectives pipeline behind the per-chunk FFN compute.
"""

import os
import sys

for _p in ("/opt/trn_rl_repo", "/root/.axon_site/_ro/trn_rl_repo"):
    if os.path.isdir(_p) and _p not in sys.path:
        sys.path.insert(0, _p)

import numpy as np
import ml_dtypes

import concourse.bass as bass  # noqa: F401
import concourse.mybir as mybir
import concourse.tile as tile
from concourse import bacc
from concourse.bass_utils import run_bass_kernel_spmd

# Problem constants (hardcoded per spec)
N_TOK = 8192
D = 2048
H = 1408
E = 8
TOP_K = 2
ROUTE_SCALE = 1.0
P = 128
KD = D // P    # 16 k-tiles over D
MH = H // P    # 11 m-tiles over H
MD = D // P    # 16 m-tiles over D (stage 2 out)
SHARD = N_TOK // E  # 1024 tokens per core

# AllGather/ReduceScatter chunking: NAG chunks of CH tokens per rank;
# each compute chunk covers all 8 ranks of one AG chunk = 8*CH tokens.
NAG = int(os.environ.get("MOE_NAG", "4"))
CH = SHARD // NAG
TCH = E * CH

# Shared-expert H-tile assignment: 11 tiles of 128 over 8 cores,
# zero-padded to SMAX=2 tiles per core so the SPMD program is uniform.
SMAX = 2
S_CNT = [2, 2, 2, 1, 1, 1, 1, 1]
S_OFF = [0, 2, 4, 6, 7, 8, 9, 10]

F32 = mybir.dt.float32
BF16 = mybir.dt.bfloat16
NP_BF16 = ml_dtypes.bfloat16
SILU = mybir.ActivationFunctionType.Silu

LAST_RESULTS = None  # BassKernelResults of the most recent run (for test.py)

SKIP_MM = bool(os.environ.get("MOE_SKIP_MM"))
SKIP_DMA = bool(os.environ.get("MOE_SKIP_DMA"))
SKIP_CC = bool(os.environ.get("MOE_SKIP_CC"))


def _subs(Tc):
    """Split Tc into matmul free-dim slices of <=512."""
    out = []
    rem = Tc
    while rem > 512:
        take = 384 if rem == 640 else 512
        out.append(take)
        rem -= take
    if rem:
        out.append(rem)
    s0 = 0
    res = []
    for s in out:
        res.append((s0, s))
        s0 += s
    return res


def _dedup_ldweights(nc):
    """Remove redundant PE Ldweights instructions (see baseline notes):
    the legalizer inserts an InstLdweights before EVERY bf16 matmul, even
    when consecutive matmuls share one stationary operand."""
    pe = mybir.EngineType.PE
    removed = 0
    for fn in nc.m.functions:
        for blk in fn.blocks:
            insts = blk.instructions
            keep = []
            last_key = None
            for inst in insts:
                if getattr(inst, "engine", None) == pe:
                    if isinstance(inst, mybir.InstLdweights):
                        ap = inst.ins[0]
                        key = (str(ap.memsetref), ap.offset, str(ap.ap),
                               str(ap.dtype), inst.is_transpose,
                               inst.tile_position, inst.perf_mode)
                        si = inst.sync_info
                        bare = si is None or (not si.on_wait and not si.on_update)
                        if bare and key == last_key:
                            removed += 1
                            continue
                        last_key = key
                    elif isinstance(inst, mybir.InstMatmult):
                        if inst.is_transpose:
                            last_key = None
                    else:
                        last_key = None
                keep.append(inst)
            if len(keep) != len(insts):
                blk.instructions = keep
    return removed


def _build_program(loop_reps=1):
    nc = bacc.Bacc("TRN2", target_bir_lowering=False, debug=False, num_devices=E)
    xs = nc.dram_tensor("xs", [NAG, KD, P, CH], BF16, kind="ExternalInput").ap()
    w1s = nc.dram_tensor("w1s", [MH, P, KD * P], BF16, kind="ExternalInput").ap()
    w3s = nc.dram_tensor("w3s", [MH, P, KD * P], BF16, kind="ExternalInput").ap()
    w2s = nc.dram_tensor("w2s", [MD, P, MH * P], BF16, kind="ExternalInput").ap()
    s1s = nc.dram_tensor("s1s", [SMAX, P, KD * P], BF16, kind="ExternalInput").ap()
    s3s = nc.dram_tensor("s3s", [SMAX, P, KD * P], BF16, kind="ExternalInput").ap()
    s2s = nc.dram_tensor("s2s", [MD, P, SMAX * P], BF16, kind="ExternalInput").ap()
    cmb = nc.dram_tensor("cmb", [1, N_TOK], BF16, kind="ExternalInput").ap()
    ys = nc.dram_tensor("ys", [NAG, MD, P, CH], BF16, kind="ExternalOutput").ap()

    rg = [list(range(E))]

    with tile.TileContext(nc) as tc:
        with tc.tile_pool(name="xpool", bufs=1) as xpool, \
             tc.tile_pool(name="wpool", bufs=3) as wpool, \
             tc.tile_pool(name="w2pool", bufs=4) as w2pool, \
             tc.tile_pool(name="gpool", bufs=1) as gpool, \
             tc.tile_pool(name="spool", bufs=3) as spool, \
             tc.tile_pool(name="ypool", bufs=3) as ypool, \
             tc.tile_pool(name="cpool", bufs=1) as cpool, \
             tc.tile_pool(name="psum", bufs=8, space="PSUM") as psum, \
             tc.tile_pool(name="dram", bufs=1, space="DRAM") as dram:

            xjc = dram.tile([NAG, KD, P, CH], BF16, name="xjc")
            xag = [dram.tile([E, KD, P, CH], BF16, name=f"xag{j}",
                             addr_space="Shared") for j in range(NAG)]
            yrs = dram.tile([NAG, E, MD, P, CH], BF16, name="yrs")
            ysb = dram.tile([NAG, MD, P, CH], BF16, name="ysb")

            # --- combine column -> [P, N_TOK] broadcast tile (once) ---
            cmb_p0 = cpool.tile([1, N_TOK], BF16, name="cmb_p0")
            SKIP_DMA or nc.sync.dma_start(cmb_p0[:], cmb)
            cmb_sb = cpool.tile([P, N_TOK], BF16, name="cmb_sb")
            SKIP_DMA or nc.gpsimd.partition_broadcast(cmb_sb[:], cmb_p0[:])

            # --- chunked AllGather of x (trigger all up front) ---
            for j in range(NAG):
                SKIP_DMA or nc.gpsimd.dma_start(xjc[j], xs[j])
                if not SKIP_CC:
                    nc.gpsimd.collective_compute(
                        "AllGather", mybir.AluOpType.bypass,
                        replica_groups=rg, ins=[xjc[j]], outs=[xag[j][:]],
                    )

            subs = _subs(TCH)

            def emit_chunk(j):
                # x chunk [P, KD*TCH]: token t = r*CH + i within the chunk
                xt = xpool.tile([P, KD * TCH], BF16, name="xt")
                for k in range(KD):
                    SKIP_DMA or nc.scalar.dma_start(
                        xt[:, k * TCH:(k + 1) * TCH].rearrange(
                            "p (r i) -> p r i", r=E),
                        xag[j][:, k].rearrange("r p i -> p r i"),
                    )
                x_tiles = [xt[:, k * TCH:(k + 1) * TCH] for k in range(KD)]

                g_tiles = []
                # expert stage 1 (scaled by combine) + shared stage 1
                for m in range(MH + SMAX):
                    is_sh = m >= MH
                    if is_sh:
                        w1d, w3d = s1s[m - MH], s3s[m - MH]
                    else:
                        w1d, w3d = w1s[m], w3s[m]
                    w1m = wpool.tile([P, KD * P], BF16, name="w1m")
                    SKIP_DMA or nc.sync.dma_start(w1m[:], w1d)
                    w3m = wpool.tile([P, KD * P], BF16, name="w3m")
                    SKIP_DMA or nc.sync.dma_start(w3m[:], w3d)
                    gm = gpool.tile([P, TCH], BF16, name=f"g{m}")
                    ps1 = [psum.tile([P, 512], F32, name="acc")[:, :sl]
                           for _, sl in subs]
                    ps3 = [psum.tile([P, 512], F32, name="acc")[:, :sl]
                           for _, sl in subs]
                    for k in range(KD):
                        w1k = w1m[:, k * P:(k + 1) * P]
                        for jj, (s0, sl) in enumerate(subs):
                            SKIP_MM or nc.tensor.matmul(
                                ps1[jj], w1k, x_tiles[k][:, s0:s0 + sl],
                                start=(k == 0), stop=(k == KD - 1),
                            )
                        w3k = w3m[:, k * P:(k + 1) * P]
                        for jj, (s0, sl) in enumerate(subs):
                            SKIP_MM or nc.tensor.matmul(
                                ps3[jj], w3k, x_tiles[k][:, s0:s0 + sl],
                                start=(k == 0), stop=(k == KD - 1),
                            )
                    for jj, (s0, sl) in enumerate(subs):
                        st = spool.tile([P, 512], BF16, name="silu")[:, :sl]
                        SKIP_MM or nc.scalar.activation(st, ps1[jj], SILU)
                        SKIP_MM or nc.vector.tensor_mul(
                            gm[:, s0:s0 + sl], st, ps3[jj])
                        if not is_sh:
                            # scale by the combine column (zero for
                            # tokens not routed to this expert)
                            SKIP_MM or nc.vector.tensor_mul(
                                gm[:, s0:s0 + sl], gm[:, s0:s0 + sl],
                                cmb_sb[:, j * TCH + s0:j * TCH + s0 + sl])
                    g_tiles.append(gm)

                # stage 2: accumulate expert (11) + shared (2) k-tiles
                for md in range(MD):
                    w2m = w2pool.tile([P, MH * P], BF16, name="w2m")
                    SKIP_DMA or nc.sync.dma_start(w2m[:], w2s[md])
                    s2m = w2pool.tile([P, SMAX * P], BF16, name="s2m")
                    SKIP_DMA or nc.sync.dma_start(s2m[:], s2s[md])
                    ym = ypool.tile([P, TCH], BF16, name="ym")
                    psy = [psum.tile([P, 512], F32, name="acc")[:, :sl]
                           for _, sl in subs]
                    nkh = MH + SMAX
                    for kh in range(nkh):
                        if kh < MH:
                            w2k = w2m[:, kh * P:(kh + 1) * P]
                        else:
                            w2k = s2m[:, (kh - MH) * P:(kh - MH + 1) * P]
                        for jj, (s0, sl) in enumerate(subs):
                            SKIP_MM or nc.tensor.matmul(
                                psy[jj], w2k, g_tiles[kh][:, s0:s0 + sl],
                                start=(kh == 0), stop=(kh == nkh - 1),
                            )
                    for jj, (s0, sl) in enumerate(subs):
                        SKIP_MM or nc.vector.tensor_copy(
                            ym[:, s0:s0 + sl], psy[jj])
                    # ym[p, r*CH+i] -> yrs[j][r, md, p, i]
                    SKIP_DMA or nc.sync.dma_start(
                        yrs[j][:, md].rearrange("r p i -> p r i"),
                        ym[:].rearrange("p (r i) -> p r i", r=E))

            def body():
                for j in range(NAG):
                    emit_chunk(j)
                    if not SKIP_CC:
                        nc.gpsimd.collective_compute(
                            "ReduceScatter", mybir.AluOpType.add,
                            replica_groups=rg, ins=[yrs[j]], outs=[ysb[j]],
                        )
                        SKIP_DMA or nc.sync.dma_start(ys[j], ysb[j])
                    else:
                        SKIP_DMA or nc.sync.dma_start(ys[j], ysb[j])

            if loop_reps > 1:
                with tc.For_i(0, loop_reps, 1):
                    body()
            else:
                body()
    nc.compile()
    if not os.environ.get("MOE_NO_LDW_DEDUP"):
        _dedup_ldweights(nc)
    return nc


def _tile_w13_stream(w):
    # [H, D] -> [MH, P, KD*P] with slab[m, p, k*P+j] = w[m*P+j, k*P+p]
    return np.ascontiguousarray(
        w.reshape(MH, P, KD, P).transpose(0, 3, 2, 1).reshape(MH, P, KD * P)
    )


def _tile_w2_stream(w):
    # [D, H] -> [MD, P, MH*P] with slab[md, p, kh*P+j] = w[md*P+j, kh*P+p]
    return np.ascontiguousarray(
        w.reshape(MD, P, MH, P).transpose(0, 3, 2, 1).reshape(MD, P, MH * P)
    )


def _tile_x_shard(xt):
    # [SHARD, D] -> [NAG, KD, P, CH]: el[j, k, p, i] = xt[j*CH+i, k*P+p]
    return np.ascontiguousarray(
        xt.reshape(NAG, CH, KD, P).transpose(0, 2, 3, 1))


def _untile_y(y):
    # [NAG, MD, P, CH] -> [SHARD, D]
    return y.transpose(0, 3, 1, 2).reshape(SHARD, D).astype(np.float32)


def prepare(x, gate_w, expert_bias, w1, w2, w3, sw1, sw2, sw3):
    """Host routing + input prep. Returns (nc, in_maps, meta)."""
    x = np.ascontiguousarray(np.asarray(x, dtype=np.float32))
    gate_w = np.asarray(gate_w, dtype=np.float32)
    expert_bias = np.asarray(expert_bias, dtype=np.float32)
    w1 = np.asarray(w1, dtype=np.float32)
    w2 = np.asarray(w2, dtype=np.float32)
    w3 = np.asarray(w3, dtype=np.float32)
    sw1 = np.asarray(sw1, dtype=np.float32)
    sw2 = np.asarray(sw2, dtype=np.float32)
    sw3 = np.asarray(sw3, dtype=np.float32)

    # ---- host router (fp32, matches reference numerics) ----
    logits = x @ gate_w.T  # [N, E] f32
    scores = np.where(
        logits >= 0,
        1.0 / (1.0 + np.exp(-logits, dtype=np.float32)),
        np.exp(logits, dtype=np.float32) / (1.0 + np.exp(logits, dtype=np.float32)),
    ).astype(np.float32)
    biased = scores + expert_bias[None, :]
    i1 = np.argmax(biased, axis=1)
    tmp = biased.copy()
    tmp[np.arange(N_TOK), i1] = -np.inf
    i2 = np.argmax(tmp, axis=1)
    s1 = scores[np.arange(N_TOK), i1]
    s2 = scores[np.arange(N_TOK), i2]
    denom = s1 + s2 + np.float32(1e-20)
    c1 = (s1 / denom * np.float32(ROUTE_SCALE)).astype(np.float32)
    c2 = (s2 / denom * np.float32(ROUTE_SCALE)).astype(np.float32)
    combine = np.zeros((N_TOK, E), dtype=np.float32)
    combine[np.arange(N_TOK), i1] = c1
    combine[np.arange(N_TOK), i2] += c2

    nc = _build_program(loop_reps=int(os.environ.get("MOE_LOOP_REPS", "1")))

    # ---- per-core inputs ----
    x_bf = x.astype(NP_BF16)
    sw1s_full = _tile_w13_stream(sw1.astype(NP_BF16))
    sw3s_full = _tile_w13_stream(sw3.astype(NP_BF16))
    sw2s_full = _tile_w2_stream(sw2.astype(NP_BF16))
    in_maps = []
    for c in range(E):
        o, n = S_OFF[c], S_CNT[c]
        s1c = np.zeros((SMAX, P, KD * P), dtype=NP_BF16)
        s3c = np.zeros((SMAX, P, KD * P), dtype=NP_BF16)
        s2c = np.zeros((MD, P, SMAX * P), dtype=NP_BF16)
        s1c[:n] = sw1s_full[o:o + n]
        s3c[:n] = sw3s_full[o:o + n]
        s2c[:, :, :n * P] = sw2s_full[:, :, o * P:(o + n) * P]
        # combine column in device token order [NAG, E, CH]
        cmb_dev = np.ascontiguousarray(
            combine[:, c].reshape(E, NAG, CH).transpose(1, 0, 2)
        ).reshape(1, N_TOK).astype(NP_BF16)
        in_maps.append({
            "xs": _tile_x_shard(x_bf[c * SHARD:(c + 1) * SHARD]),
            "w1s": _tile_w13_stream(w1[c].astype(NP_BF16)),
            "w3s": _tile_w13_stream(w3[c].astype(NP_BF16)),
            "w2s": _tile_w2_stream(w2[c].astype(NP_BF16)),
            "s1s": s1c,
            "s3s": s3c,
            "s2s": s2c,
            "cmb": cmb_dev,
        })

    meta = None
    return nc, in_maps, meta


def combine(meta, results):
    """Assemble per-core output shards into the final [N, D] array."""
    out = np.empty((N_TOK, D), dtype=np.float32)
    for c in range(E):
        out[c * SHARD:(c + 1) * SHARD] = _untile_y(results[c]["ys"])
    return out


def kernel(x, gate_w, expert_bias, w1, w2, w3, sw1, sw2, sw3):
    nc, in_maps, meta = prepare(x, gate_w, expert_bias, w1, w2, w3, sw1, sw2, sw3)
    global LAST_RESULTS
    res = run_bass_kernel_spmd(nc, in_maps, core_ids=list(range(E)))
    LAST_RESULTS = res
    return combine(meta, res.results)
